# revision 43
# baseline (speedup 1.0000x reference)
"""Trainium2 Bass kernel for nn_CrossAttention (B=8, N1=64, N2=4096, C=768, H=12).

Strategy: data-parallel over batch across 8 NeuronCores (one item per core,
no collectives). All activations kept transposed (channels on partitions,
tokens on the free dim) so every matmul contracts over SBUF partitions.

Key restructurings (exploiting that the combine with v is ELEMENTWISE):

  1. scores_h = q_h @ k_h^T = (q_h @ W_k_h) @ yT = A_h @ yT.  A is a tiny
     [768,768] input-dependent precompute done on the host; scores then
     contract over the full 768 channels with the same moving operand (yT)
     as the v-projection — k is never materialized.
  2. softmax normalization is deferred: U_h = exp(s_h) * vT_h is accumulated
     unnormalized; row-sums S come free via ACT's fused accum_out; 1/S is
     folded into the projection weights (O(C^2), not O(C*N2)).
  3. fp8e4 DoubleRow matmuls (2 stacked K-tiles per pass, 0.5 cycles/row)
     for the two y-streaming contractions:
       - scores: plain e4m3 (A*64, yT) — softmax's small logit scale damps
         the quantization error ~3x.
       - vproj: 3-term hi/lo split (Wv_hi*y_hi + Wv_lo*y_hi + Wv_hi*y_lo),
         residuals stored at the SAME scale as hi (e4m3 subnormals carry
         them) so all terms accumulate in one PSUM group.
     The output projection stays bf16: fp8 there would need U split into
     fp8 hi/lo pairs, whose ~26us of extra vector work exceeds the 15us of
     PE time saved and starves the chunk pipeline.

Schedule notes (the modeled machine):
  - Engines dispatch from in-order queues with a small wait-queue, so
    low-urgency work can delay urgent drains; PSUM pool slots are the
    coupling.  A stalled PE also resets the modeled p-state ramp (the next
    matmuls run 2-3.7x slow), so gaps are doubly expensive.
  - Per chunk the PE interleaves one score pair-tile with one vproj tile;
    exps drain score PSUM during the vproj groups.  Chunk 0 runs all its
    scores first (they only need A + y0_hi, which arrive first).
  - The e*v multiplies for chunk c are issued at the top of chunk c+1's
    steps (producer-first order, keeps e-tile reuse clean).
  - The 1/S fold for pair g is issued right after chunk 3's scores for g,
    so the last fold lands while vproj(3) still owns the PE.
  - The HWDGE dispatch (~0.63us) and the modeled DMA device are serial;
    the prologue is explicitly ordered [A-cols, y0_hi k-pairs, rest of A,
    Wv_hi, Wv_lo, y0_lo] and all non-critical DMAs are dep-pinned behind
    chunk exps so the scheduler cannot hoist them into that stream.
  - A short warmup matmul chain burns the DMA-prologue wait so the real
    work starts at full PE clock.

Softmax statistics and PSUM accumulation are f32 throughout.
"""

import numpy as np
import ml_dtypes

from bass_rust import add_dep_helper

import concourse.bass as bass
import concourse.mybir as mybir
import concourse.tile as tile
from concourse import bacc
from concourse.bass_utils import run_bass_kernel_spmd

BF16 = mybir.dt.bfloat16
F32 = mybir.dt.float32
E4 = mybir.dt.float8e4
NPE4 = ml_dtypes.float8_e4m3
DR = mybir.MatmulPerfMode.DoubleRow

B, N1, N2, C, H = 8, 64, 4096, 768, 12
HD = C // H              # 64
SCALE = HD ** -0.5       # 1/8
CT = C // 128            # 6 partition tiles of channels
KP = CT // 2             # 3 DoubleRow k-tile pairs
CHUNK = 1024             # tokens per streamed chunk
NCH = N2 // CHUNK        # 4 chunks

SA = 64.0                # fp8 scale for A
SW = 32.0                # fp8 scale for Wv

_CACHE = {}


def _build():
    nc = bacc.Bacc("TRN2", target_bir_lowering=False, debug=False)

    # AT[c_in, (h,d)] = A^T quantized e4m3 * SA (scores lhsT; host precompute)
    AT_d = nc.dram_tensor("AT", [C, C], E4, kind="ExternalInput")
    yh_d = nc.dram_tensor("yh", [C, N2], E4, kind="ExternalInput")
    yl_d = nc.dram_tensor("yl", [C, N2], E4, kind="ExternalInput")
    # W_v^T * SW hi/lo (residual at the SAME scale: e4m3 subnormals)
    wvh_d = nc.dram_tensor("wvh", [C, C], E4, kind="ExternalInput")
    wvl_d = nc.dram_tensor("wvl", [C, C], E4, kind="ExternalInput")
    wprojT_d = nc.dram_tensor("wprojT", [C, C], BF16, kind="ExternalInput")
    bproj_d = nc.dram_tensor("bproj", [C, 1], F32, kind="ExternalInput")
    outT_d = nc.dram_tensor("outT", [C, N2], BF16, kind="ExternalOutput")

    def t6(ap):  # [768, X] dram view -> [128, 6, X] partition-tiled view
        return ap.rearrange("(t p) c -> p t c", p=128)

    with tile.TileContext(nc) as tc:
        with (
            tc.tile_pool(name="persist", bufs=1) as pp,
            tc.tile_pool(name="work", bufs=2) as wp,
            tc.tile_pool(name="psum", bufs=2, space=bass.MemorySpace.PSUM) as psp,
        ):
            # ---- persistent tiles (partition-tiled: [:, kk, :] = rows of 128)
            AT_sb = pp.tile([128, CT, C], E4, name="AT", tag="AT")
            wvh_sb = pp.tile([128, CT, C], E4, name="wvh", tag="wvh")
            wvl_sb = pp.tile([128, CT, C], E4, name="wvl", tag="wvl")
            wp_sb = pp.tile([128, CT, C], BF16, name="wpr", tag="wpr")
            wps_sb = pp.tile([128, CT, C], BF16, name="wps", tag="wps")
            bias_sb = pp.tile([128, CT, 1], F32, name="biass", tag="biass")
            # U = exp(s)*v, bf16, k-tiled on dim1 (one tile: contiguous kk)
            U_sb = pp.tile([128, CT, N2], BF16, name="U", tag="U")
            S_parts = [pp.tile([128, NCH], F32, name=f"Sp{g}", tag=f"Sp{g}")
                       for g in range(CT)]
            # ---- PE warmup --------------------------------------------------
            warm = pp.tile([128, 512], BF16, name="warm", tag="warm")
            nc.gpsimd.memset(warm[:], 0.0)
            for _ in range(9):
                psw = psp.tile([128, 512], F32, name="psw", tag="pskv", bufs=4)
                nc.tensor.matmul(psw[:], warm[:, 0:128], warm[:],
                                 start=True, stop=True)

            zbias = pp.tile([128, 1], F32, name="zbias", tag="zbias")
            nc.gpsimd.memset(zbias[:], 0.0)

            # ---- prologue DMAs (explicit global order) ----------------------
            nc.sync.dma_start(AT_sb[:, :, 0:512], t6(AT_d[:, 0:512]))

            def chunk_tiles():
                yh_c = wp.tile([128, CT, CHUNK], E4, name="yhc", tag="yhc",
                               bufs=2)
                yl_c = wp.tile([128, CT, CHUNK], E4, name="ylc", tag="ylc",
                               bufs=2)
                vT_c = [wp.tile([128, CHUNK], BF16, name=f"vTc{m}",
                                tag=f"vTc{m}", bufs=2) for m in range(CT)]
                return (yh_c, yl_c), vT_c

            y0, vT0 = chunk_tiles()
            for kp in range(KP):
                nc.sync.dma_start(y0[0][:, 2 * kp:2 * kp + 2, :],
                                  t6(yh_d[:, 0:CHUNK])[:, 2 * kp:2 * kp + 2, :])
            nc.sync.dma_start(wvh_sb[:], t6(wvh_d[:, :]))
            nc.sync.dma_start(AT_sb[:, :, 512:768], t6(AT_d[:, 512:768]))
            nc.sync.dma_start(wvl_sb[:], t6(wvl_d[:, :]))
            for kp in range(KP):
                nc.sync.dma_start(y0[1][:, 2 * kp:2 * kp + 2, :],
                                  t6(yl_d[:, 0:CHUNK])[:, 2 * kp:2 * kp + 2, :])

            def chunk_dma(c, y_c, anchor):
                # `anchor` pins the scalar-queue transfer behind chunk c-1's
                # first exp so the scheduler can't hoist it into the critical
                # prologue stream (the modeled DMA device is serial).
                tok = slice(CHUNK * c, CHUNK * (c + 1))
                yh_c, yl_c = y_c
                nc.sync.dma_start(yh_c[:], t6(yh_d[:, tok]))
                d = nc.scalar.dma_start(yl_c[:], t6(yl_d[:, tok]))
                add_dep_helper(d.ins, anchor.ins,
                               reason="defer lo-stream behind prologue")

            def scores_g(c, y_c, g, es, einsts):
                """One score pair-tile: 2 psum half-groups, one fused exp."""
                yh_c, _ = y_c
                pss = psp.tile([128, CHUNK], F32, name="pss", tag="pss",
                               bufs=2)
                for kp in range(KP):
                    for hf in range(2):
                        nc.tensor.matmul(
                            pss[:, 512 * hf:512 * (hf + 1)],
                            AT_sb[:, 2 * kp:2 * kp + 2, 128 * g:128 * (g + 1)],
                            yh_c[:, 2 * kp:2 * kp + 2,
                                 512 * hf:512 * (hf + 1)],
                            start=(kp == 0), stop=(kp == KP - 1),
                            perf_mode=DR,
                        )
                e_sb = wp.tile([128, CHUNK], BF16, name="e_sb", tag="e_sb",
                               bufs=6)
                einsts[g] = nc.scalar.activation(
                    e_sb[:], pss[:],
                    mybir.ActivationFunctionType.Exp,
                    bias=zbias[:], scale=1.0 / SA,
                    accum_out=S_parts[g][:, c:c + 1])
                es[g] = e_sb

            def scores_mul_g(c, es, vT_c, g):
                nc.vector.tensor_mul(
                    U_sb[:, g, CHUNK * c:CHUNK * (c + 1)],
                    es[g][:], vT_c[g][:])

            def vproj_m(y_c, vT_c, m):
                yh_c, yl_c = y_c
                seq = [(W, Y, kp)
                       for (W, Y) in ((wvh_sb, yh_c), (wvl_sb, yh_c),
                                      (wvh_sb, yl_c))
                       for kp in range(KP)]
                ph = [psp.tile([128, 512], F32, name="pskv", tag="pskv",
                               bufs=4) for _ in range(2)]
                for i, (W, Y, kp) in enumerate(seq):
                    for hf in range(2):  # same lhsT twice: LDW amortized
                        nc.tensor.matmul(
                            ph[hf][:],
                            W[:, 2 * kp:2 * kp + 2, 128 * m:128 * (m + 1)],
                            Y[:, 2 * kp:2 * kp + 2, 512 * hf:512 * (hf + 1)],
                            start=(i == 0), stop=(i == len(seq) - 1),
                            perf_mode=DR,
                        )
                for hf in range(2):
                    if m < 3:
                        nc.scalar.activation(
                            vT_c[m][:, 512 * hf:512 * (hf + 1)], ph[hf][:],
                            mybir.ActivationFunctionType.Copy,
                            bias=0.0, scale=1.0 / SW)
                    else:
                        nc.vector.tensor_scalar_mul(
                            vT_c[m][:, 512 * hf:512 * (hf + 1)], ph[hf][:],
                            1.0 / SW)

            def fold(g):
                # wps[:, g, :] = wp[:, g, :] / S_g  (1/S folded into weights)
                S_tot = wp.tile([128, 1], F32, name="S_tot", tag="S_tot",
                                bufs=2)
                nc.vector.tensor_reduce(S_tot[:], S_parts[g][:],
                                        axis=mybir.AxisListType.X,
                                        op=mybir.AluOpType.add)
                R_g = wp.tile([128, 1], F32, name="R_g", tag="R_g", bufs=2)
                nc.vector.reciprocal(R_g[:], S_tot[:])
                nc.vector.tensor_scalar_mul(wps_sb[:, g, :], wp_sb[:, g, :],
                                            R_g[:])

            # ---- stream over token chunks -----------------------------------
            y_c, vT_c = y0, vT0
            prev = None
            for c in range(NCH):
                es = [None] * CT
                einsts = [None] * CT
                for g in range(CT):
                    if prev is not None:
                        scores_mul_g(c - 1, prev[0], prev[1], g)
                    scores_g(c, y_c, g, es, einsts)
                    if c == 0:
                        continue  # chunk 0: all scores first, vproj after
                    if g == 0 and c + 1 < NCH:
                        y_next, vT_next = chunk_tiles()
                        chunk_dma(c + 1, y_next, einsts[0])
                    if c == NCH - 1:
                        fold(g)
                    vproj_m(y_c, vT_c, g)
                if c == 0:
                    y_next, vT_next = chunk_tiles()
                    chunk_dma(1, y_next, einsts[0])
                    for m in range(CT):
                        vproj_m(y_c, vT_c, m)
                    d1 = nc.scalar.dma_start(wp_sb[:], t6(wprojT_d[:, :]))
                    d2 = nc.scalar.dma_start(bias_sb[:], t6(bproj_d[:, :]))
                    for d in (d1, d2):
                        add_dep_helper(d.ins, einsts[-1].ins,
                                       reason="defer proj weights")
                prev = (es, vT_c)
                if c + 1 < NCH:
                    y_c, vT_c = y_next, vT_next
            for g in range(CT):
                scores_mul_g(NCH - 1, prev[0], prev[1], g)

            # ---- outT = W_proj_scaled @ U + b -------------------------------
            # n outer so output stores batch per chunk; the last chunk stores
            # per half-tile to keep the kernel tail short.
            for n in range(NCH):
                tok = slice(CHUNK * n, CHUNK * (n + 1))
                last = (n == NCH - 1)
                outc = None
                for m in range(CT):
                    if m % 3 == 0 and not last:
                        outc = wp.tile([128, 3, CHUNK], BF16, name="outc",
                                       tag="outc", bufs=3)
                    outm = None
                    if last:
                        outm = wp.tile([128, CHUNK], BF16, name="outm",
                                       tag="outm", bufs=2)
                    for hf in range(2):
                        psq = psp.tile([128, 512], F32, name="psq", tag="pskv",
                                       bufs=4)
                        for kk in range(CT):
                            nc.tensor.matmul(
                                psq[:],
                                wps_sb[:, kk, 128 * m:128 * (m + 1)],
                                U_sb[:, kk, CHUNK * n + 512 * hf:
                                     CHUNK * n + 512 * (hf + 1)],
                                start=(kk == 0), stop=(kk == CT - 1),
                            )
                        half = slice(512 * hf, 512 * (hf + 1))
                        dst = outm[:, half] if last else outc[:, m % 3, half]
                        if (m + hf) % 2 == 0:
                            nc.scalar.add(dst, psq[:], add=bias_sb[:, m, :])
                        else:
                            nc.vector.tensor_scalar_add(dst, psq[:],
                                                        bias_sb[:, m, :])
                        if last:
                            nc.sync.dma_start(
                                outT_d[128 * m:128 * (m + 1),
                                       CHUNK * n + 512 * hf:
                                       CHUNK * n + 512 * (hf + 1)],
                                outm[:, half])
                    if not last and m % 3 == 2:
                        h3 = m // 3
                        nc.sync.dma_start(
                            outT_d[384 * h3:384 * (h3 + 1), tok].rearrange(
                                "(t p) c -> p t c", p=128),
                            outc[:])

    nc.compile()
    return nc


def kernel(x, y, W_qkv, W_proj, b_proj):
    if "nc" not in _CACHE:
        _CACHE["nc"] = _build()
    nc = _CACHE["nc"]
    in_maps = make_in_maps(x, y, W_qkv, W_proj, b_proj)
    # The axon-tunneled devices occasionally fail one execution with a
    # transient NRT_EXEC_UNIT_UNRECOVERABLE; a clean retry succeeds.
    last_err = None
    for attempt in range(3):
        try:
            res = run_bass_kernel_spmd(nc, in_maps, core_ids=list(range(B)))
            break
        except Exception as e:  # noqa: BLE001
            last_err = e
            import time
            time.sleep(2.0 * (attempt + 1))
    else:
        raise last_err
    out = np.empty((B, N2, C), np.float32)
    for i in range(B):
        out[i] = res.results[i]["outT"].T
    return out


def _hi_lo(a):
    """e4m3 hi + residual at the SAME scale (subnormals carry the tail)."""
    hi = np.asarray(a, NPE4)
    lo = np.asarray(a - hi.astype(np.float32), NPE4)
    return hi, lo


def make_in_maps(x, y, W_qkv, W_proj, b_proj):
    bf = ml_dtypes.bfloat16
    x = np.asarray(x, np.float32)
    y = np.asarray(y, np.float32)
    W_qkv = np.asarray(W_qkv, np.float32)
    Wq, Wk, Wv = W_qkv[:C], W_qkv[C:2 * C], W_qkv[2 * C:]

    # A[b, (h,d), c] = sum_j q[b,d,(h,j)] * Wk[(h,j), c],  q = x @ Wq^T * 1/8
    q = np.einsum("bnc,jc->bnj", x, Wq, optimize=True) * SCALE  # [B, N1, C]
    A = np.einsum("bnhj,hjc->bhnc",
                  q.reshape(B, N1, H, HD),
                  Wk.reshape(H, HD, C), optimize=True).reshape(B, C, C)
    AT = np.ascontiguousarray(A.transpose(0, 2, 1)) * SA        # [B, c, (h,d)]

    wvh, wvl = _hi_lo(np.ascontiguousarray(Wv.T) * SW)
    wprojT = np.ascontiguousarray(np.asarray(W_proj, np.float32).T).astype(bf)
    bproj = np.asarray(b_proj, np.float32).reshape(C, 1)

    in_maps = []
    for i in range(B):
        yT = np.ascontiguousarray(y[i].T)
        yh, yl = _hi_lo(yT)
        in_maps.append({
            "AT": np.asarray(AT[i], NPE4),
            "yh": yh,
            "yl": yl,
            "wvh": wvh,
            "wvl": wvl,
            "wprojT": wprojT,
            "bproj": bproj,
        })
    return in_maps


# revision 44
# speedup vs baseline: 1.0008x; 1.0008x over previous
"""Trainium2 Bass kernel for nn_CrossAttention (B=8, N1=64, N2=4096, C=768, H=12).

Strategy: data-parallel over batch across 8 NeuronCores (one item per core,
no collectives). All activations kept transposed (channels on partitions,
tokens on the free dim) so every matmul contracts over SBUF partitions.

Key restructurings (exploiting that the combine with v is ELEMENTWISE):

  1. scores_h = q_h @ k_h^T = (q_h @ W_k_h) @ yT = A_h @ yT.  A is a tiny
     [768,768] input-dependent precompute done on the host; scores then
     contract over the full 768 channels with the same moving operand (yT)
     as the v-projection — k is never materialized.
  2. softmax normalization is deferred: U_h = exp(s_h) * vT_h is accumulated
     unnormalized; row-sums S come free via ACT's fused accum_out; 1/S is
     folded into the projection weights (O(C^2), not O(C*N2)).
  3. fp8e4 DoubleRow matmuls (2 stacked K-tiles per pass, 0.5 cycles/row)
     for the two y-streaming contractions:
       - scores: plain e4m3 (A*64, yT) — softmax's small logit scale damps
         the quantization error ~3x.
       - vproj: 3-term hi/lo split (Wv_hi*y_hi + Wv_lo*y_hi + Wv_hi*y_lo),
         residuals stored at the SAME scale as hi (e4m3 subnormals carry
         them) so all terms accumulate in one PSUM group.
     The output projection stays bf16: fp8 there would need U split into
     fp8 hi/lo pairs, whose ~26us of extra vector work exceeds the 15us of
     PE time saved and starves the chunk pipeline.

Schedule notes (the modeled machine):
  - Engines dispatch from in-order queues with a small wait-queue, so
    low-urgency work can delay urgent drains; PSUM pool slots are the
    coupling.  A stalled PE also resets the modeled p-state ramp (the next
    matmuls run 2-3.7x slow), so gaps are doubly expensive.
  - Per chunk the PE interleaves one score pair-tile with one vproj tile;
    exps drain score PSUM during the vproj groups.  Chunk 0 runs all its
    scores first (they only need A + y0_hi, which arrive first).
  - The e*v multiplies for chunk c are issued at the top of chunk c+1's
    steps (producer-first order, keeps e-tile reuse clean).
  - The 1/S fold for pair g is issued right after chunk 3's scores for g,
    so the last fold lands while vproj(3) still owns the PE.
  - The HWDGE dispatch (~0.63us) and the modeled DMA device are serial;
    the prologue is explicitly ordered [A-cols, y0_hi k-pairs, rest of A,
    Wv_hi, Wv_lo, y0_lo] and all non-critical DMAs are dep-pinned behind
    chunk exps so the scheduler cannot hoist them into that stream.
  - A short warmup matmul chain burns the DMA-prologue wait so the real
    work starts at full PE clock.

Softmax statistics and PSUM accumulation are f32 throughout.
"""

import numpy as np
import ml_dtypes

from bass_rust import add_dep_helper

import concourse.bass as bass
import concourse.mybir as mybir
import concourse.tile as tile
from concourse import bacc
from concourse.bass_utils import run_bass_kernel_spmd

BF16 = mybir.dt.bfloat16
F32 = mybir.dt.float32
E4 = mybir.dt.float8e4
NPE4 = ml_dtypes.float8_e4m3
DR = mybir.MatmulPerfMode.DoubleRow

B, N1, N2, C, H = 8, 64, 4096, 768, 12
HD = C // H              # 64
SCALE = HD ** -0.5       # 1/8
CT = C // 128            # 6 partition tiles of channels
KP = CT // 2             # 3 DoubleRow k-tile pairs
CHUNK = 1024             # tokens per streamed chunk
NCH = N2 // CHUNK        # 4 chunks

SA = 64.0                # fp8 scale for A
SW = 32.0                # fp8 scale for Wv

_CACHE = {}


def _build():
    nc = bacc.Bacc("TRN2", target_bir_lowering=False, debug=False)

    # AT[c_in, (h,d)] = A^T quantized e4m3 * SA (scores lhsT; host precompute)
    AT_d = nc.dram_tensor("AT", [C, C], E4, kind="ExternalInput")
    yh_d = nc.dram_tensor("yh", [C, N2], E4, kind="ExternalInput")
    yl_d = nc.dram_tensor("yl", [C, N2], E4, kind="ExternalInput")
    # W_v^T * SW hi/lo (residual at the SAME scale: e4m3 subnormals)
    wvh_d = nc.dram_tensor("wvh", [C, C], E4, kind="ExternalInput")
    wvl_d = nc.dram_tensor("wvl", [C, C], E4, kind="ExternalInput")
    wprojT_d = nc.dram_tensor("wprojT", [C, C], BF16, kind="ExternalInput")
    bproj_d = nc.dram_tensor("bproj", [C, 1], F32, kind="ExternalInput")
    outT_d = nc.dram_tensor("outT", [C, N2], BF16, kind="ExternalOutput")

    def t6(ap):  # [768, X] dram view -> [128, 6, X] partition-tiled view
        return ap.rearrange("(t p) c -> p t c", p=128)

    with tile.TileContext(nc) as tc:
        with (
            tc.tile_pool(name="persist", bufs=1) as pp,
            tc.tile_pool(name="work", bufs=2) as wp,
            tc.tile_pool(name="psum", bufs=2, space=bass.MemorySpace.PSUM) as psp,
        ):
            # ---- persistent tiles (partition-tiled: [:, kk, :] = rows of 128)
            AT_sb = pp.tile([128, CT, C], E4, name="AT", tag="AT")
            wvh_sb = pp.tile([128, CT, C], E4, name="wvh", tag="wvh")
            wvl_sb = pp.tile([128, CT, C], E4, name="wvl", tag="wvl")
            wp_sb = pp.tile([128, CT, C], BF16, name="wpr", tag="wpr")
            wps_sb = pp.tile([128, CT, C], BF16, name="wps", tag="wps")
            bias_sb = pp.tile([128, CT, 1], F32, name="biass", tag="biass")
            # U = exp(s)*v, bf16, k-tiled on dim1 (one tile: contiguous kk)
            U_sb = pp.tile([128, CT, N2], BF16, name="U", tag="U")
            S_parts = [pp.tile([128, NCH], F32, name=f"Sp{g}", tag=f"Sp{g}")
                       for g in range(CT)]
            # ---- PE warmup --------------------------------------------------
            warm = pp.tile([128, 512], BF16, name="warm", tag="warm")
            nc.gpsimd.memset(warm[:], 0.0)
            for _ in range(9):
                psw = psp.tile([128, 512], F32, name="psw", tag="pskv", bufs=4)
                nc.tensor.matmul(psw[:], warm[:, 0:128], warm[:],
                                 start=True, stop=True)

            zbias = pp.tile([128, 1], F32, name="zbias", tag="zbias")
            nc.gpsimd.memset(zbias[:], 0.0)

            # ---- prologue DMAs (explicit global order) ----------------------
            nc.sync.dma_start(AT_sb[:, :, 0:512], t6(AT_d[:, 0:512]))

            def chunk_tiles():
                yh_c = wp.tile([128, CT, CHUNK], E4, name="yhc", tag="yhc",
                               bufs=2)
                yl_c = wp.tile([128, CT, CHUNK], E4, name="ylc", tag="ylc",
                               bufs=2)
                vT_c = [wp.tile([128, CHUNK], BF16, name=f"vTc{m}",
                                tag=f"vTc{m}", bufs=2) for m in range(CT)]
                return (yh_c, yl_c), vT_c

            y0, vT0 = chunk_tiles()
            for kp in range(KP):
                nc.sync.dma_start(y0[0][:, 2 * kp:2 * kp + 2, :],
                                  t6(yh_d[:, 0:CHUNK])[:, 2 * kp:2 * kp + 2, :])
            nc.sync.dma_start(wvh_sb[:], t6(wvh_d[:, :]))
            nc.sync.dma_start(AT_sb[:, :, 512:768], t6(AT_d[:, 512:768]))
            nc.sync.dma_start(wvl_sb[:], t6(wvl_d[:, :]))
            for kp in range(KP):
                nc.sync.dma_start(y0[1][:, 2 * kp:2 * kp + 2, :],
                                  t6(yl_d[:, 0:CHUNK])[:, 2 * kp:2 * kp + 2, :])

            def chunk_dma(c, y_c, anchor):
                # `anchor` pins the scalar-queue transfer behind chunk c-1's
                # first exp so the scheduler can't hoist it into the critical
                # prologue stream (the modeled DMA device is serial).
                tok = slice(CHUNK * c, CHUNK * (c + 1))
                yh_c, yl_c = y_c
                nc.sync.dma_start(yh_c[:], t6(yh_d[:, tok]))
                d = nc.scalar.dma_start(yl_c[:], t6(yl_d[:, tok]))
                add_dep_helper(d.ins, anchor.ins,
                               reason="defer lo-stream behind prologue")

            def scores_g(c, y_c, g, es, einsts):
                """One score pair-tile: 2 psum half-groups, one fused exp."""
                yh_c, _ = y_c
                pss = psp.tile([128, CHUNK], F32, name="pss", tag="pss",
                               bufs=2)
                for kp in range(KP):
                    for hf in range(2):
                        nc.tensor.matmul(
                            pss[:, 512 * hf:512 * (hf + 1)],
                            AT_sb[:, 2 * kp:2 * kp + 2, 128 * g:128 * (g + 1)],
                            yh_c[:, 2 * kp:2 * kp + 2,
                                 512 * hf:512 * (hf + 1)],
                            start=(kp == 0), stop=(kp == KP - 1),
                            perf_mode=DR,
                        )
                e_sb = wp.tile([128, CHUNK], BF16, name="e_sb", tag="e_sb",
                               bufs=6)
                einsts[g] = nc.scalar.activation(
                    e_sb[:], pss[:],
                    mybir.ActivationFunctionType.Exp,
                    bias=zbias[:], scale=1.0 / SA,
                    accum_out=S_parts[g][:, c:c + 1])
                es[g] = e_sb

            def scores_mul_g(c, es, vT_c, g):
                nc.vector.tensor_mul(
                    U_sb[:, g, CHUNK * c:CHUNK * (c + 1)],
                    es[g][:], vT_c[g][:])

            def vproj_m(y_c, vT_c, m):
                yh_c, yl_c = y_c
                seq = [(W, Y, kp)
                       for (W, Y) in ((wvh_sb, yh_c), (wvl_sb, yh_c),
                                      (wvh_sb, yl_c))
                       for kp in range(KP)]
                ph = [psp.tile([128, 512], F32, name="pskv", tag="pskv",
                               bufs=4) for _ in range(2)]
                for i, (W, Y, kp) in enumerate(seq):
                    for hf in range(2):  # same lhsT twice: LDW amortized
                        nc.tensor.matmul(
                            ph[hf][:],
                            W[:, 2 * kp:2 * kp + 2, 128 * m:128 * (m + 1)],
                            Y[:, 2 * kp:2 * kp + 2, 512 * hf:512 * (hf + 1)],
                            start=(i == 0), stop=(i == len(seq) - 1),
                            perf_mode=DR,
                        )
                for hf in range(2):
                    if m < 3:
                        nc.scalar.activation(
                            vT_c[m][:, 512 * hf:512 * (hf + 1)], ph[hf][:],
                            mybir.ActivationFunctionType.Copy,
                            bias=0.0, scale=1.0 / SW)
                    else:
                        nc.vector.tensor_scalar_mul(
                            vT_c[m][:, 512 * hf:512 * (hf + 1)], ph[hf][:],
                            1.0 / SW)

            def fold(g):
                # wps[:, g, :] = wp[:, g, :] / S_g  (1/S folded into weights)
                S_tot = wp.tile([128, 1], F32, name="S_tot", tag="S_tot",
                                bufs=2)
                nc.vector.tensor_reduce(S_tot[:], S_parts[g][:],
                                        axis=mybir.AxisListType.X,
                                        op=mybir.AluOpType.add)
                R_g = wp.tile([128, 1], F32, name="R_g", tag="R_g", bufs=2)
                nc.vector.reciprocal(R_g[:], S_tot[:])
                nc.vector.tensor_scalar_mul(wps_sb[:, g, :], wp_sb[:, g, :],
                                            R_g[:])

            # ---- stream over token chunks -----------------------------------
            y_c, vT_c = y0, vT0
            prev = None
            for c in range(NCH):
                es = [None] * CT
                einsts = [None] * CT
                for g in range(CT):
                    if prev is not None:
                        scores_mul_g(c - 1, prev[0], prev[1], g)
                    scores_g(c, y_c, g, es, einsts)
                    if c == 0:
                        continue  # chunk 0: all scores first, vproj after
                    if g == 0 and c + 1 < NCH:
                        y_next, vT_next = chunk_tiles()
                        chunk_dma(c + 1, y_next, einsts[0])
                    if c == NCH - 1:
                        fold(g)
                    vproj_m(y_c, vT_c, g)
                if c == 0:
                    y_next, vT_next = chunk_tiles()
                    chunk_dma(1, y_next, einsts[0])
                    for m in range(CT):
                        vproj_m(y_c, vT_c, m)
                    d1 = nc.scalar.dma_start(wp_sb[:], t6(wprojT_d[:, :]))
                    d2 = nc.scalar.dma_start(bias_sb[:], t6(bproj_d[:, :]))
                    for d in (d1, d2):
                        add_dep_helper(d.ins, einsts[-1].ins,
                                       reason="defer proj weights")
                prev = (es, vT_c)
                if c + 1 < NCH:
                    y_c, vT_c = y_next, vT_next
            for g in range(CT):
                scores_mul_g(NCH - 1, prev[0], prev[1], g)

            # ---- outT = W_proj_scaled @ U + b -------------------------------
            # n outer so output stores batch per chunk; the last chunk stores
            # per half-tile to keep the kernel tail short.
            for n in range(NCH):
                tok = slice(CHUNK * n, CHUNK * (n + 1))
                last = (n == NCH - 1)
                outc = None
                for m in range(CT):
                    if m % 3 == 0 and not last:
                        outc = wp.tile([128, 3, CHUNK], BF16, name="outc",
                                       tag="outc", bufs=3)
                    outm = None
                    if last:
                        outm = wp.tile([128, CHUNK], BF16, name="outm",
                                       tag="outm", bufs=2)
                    for hf in range(2):
                        # the kernel's very last tile finishes as two 256-col
                        # pieces: the final add+store+sem chain is ~0.6us
                        # shorter that way.
                        fine = last and m == CT - 1 and hf == 1
                        pieces = ((0, 256), (256, 512)) if fine else ((0, 512),)
                        for lo, hi in pieces:
                            psq = psp.tile([128, 512], F32, name="psq",
                                           tag="pskv", bufs=4)
                            for kk in range(CT):
                                nc.tensor.matmul(
                                    psq[:, 0:hi - lo],
                                    wps_sb[:, kk, 128 * m:128 * (m + 1)],
                                    U_sb[:, kk, CHUNK * n + 512 * hf + lo:
                                         CHUNK * n + 512 * hf + hi],
                                    start=(kk == 0), stop=(kk == CT - 1),
                                )
                            piece = slice(512 * hf + lo, 512 * hf + hi)
                            dst = (outm[:, piece] if last
                                   else outc[:, m % 3, piece])
                            if (m + hf) % 2 == 0:
                                nc.scalar.add(dst, psq[:, 0:hi - lo],
                                              add=bias_sb[:, m, :])
                            else:
                                nc.vector.tensor_scalar_add(
                                    dst, psq[:, 0:hi - lo], bias_sb[:, m, :])
                            if last:
                                nc.sync.dma_start(
                                    outT_d[128 * m:128 * (m + 1),
                                           CHUNK * n + 512 * hf + lo:
                                           CHUNK * n + 512 * hf + hi],
                                    outm[:, piece])
                    if not last and m % 3 == 2:
                        h3 = m // 3
                        nc.sync.dma_start(
                            outT_d[384 * h3:384 * (h3 + 1), tok].rearrange(
                                "(t p) c -> p t c", p=128),
                            outc[:])

    nc.compile()
    return nc


def kernel(x, y, W_qkv, W_proj, b_proj):
    if "nc" not in _CACHE:
        _CACHE["nc"] = _build()
    nc = _CACHE["nc"]
    in_maps = make_in_maps(x, y, W_qkv, W_proj, b_proj)
    # The axon-tunneled devices occasionally fail one execution with a
    # transient NRT_EXEC_UNIT_UNRECOVERABLE; a clean retry succeeds.
    last_err = None
    for attempt in range(3):
        try:
            res = run_bass_kernel_spmd(nc, in_maps, core_ids=list(range(B)))
            break
        except Exception as e:  # noqa: BLE001
            last_err = e
            import time
            time.sleep(2.0 * (attempt + 1))
    else:
        raise last_err
    out = np.empty((B, N2, C), np.float32)
    for i in range(B):
        out[i] = res.results[i]["outT"].T
    return out


def _hi_lo(a):
    """e4m3 hi + residual at the SAME scale (subnormals carry the tail)."""
    hi = np.asarray(a, NPE4)
    lo = np.asarray(a - hi.astype(np.float32), NPE4)
    return hi, lo


def make_in_maps(x, y, W_qkv, W_proj, b_proj):
    bf = ml_dtypes.bfloat16
    x = np.asarray(x, np.float32)
    y = np.asarray(y, np.float32)
    W_qkv = np.asarray(W_qkv, np.float32)
    Wq, Wk, Wv = W_qkv[:C], W_qkv[C:2 * C], W_qkv[2 * C:]

    # A[b, (h,d), c] = sum_j q[b,d,(h,j)] * Wk[(h,j), c],  q = x @ Wq^T * 1/8
    q = np.einsum("bnc,jc->bnj", x, Wq, optimize=True) * SCALE  # [B, N1, C]
    A = np.einsum("bnhj,hjc->bhnc",
                  q.reshape(B, N1, H, HD),
                  Wk.reshape(H, HD, C), optimize=True).reshape(B, C, C)
    AT = np.ascontiguousarray(A.transpose(0, 2, 1)) * SA        # [B, c, (h,d)]

    wvh, wvl = _hi_lo(np.ascontiguousarray(Wv.T) * SW)
    wprojT = np.ascontiguousarray(np.asarray(W_proj, np.float32).T).astype(bf)
    bproj = np.asarray(b_proj, np.float32).reshape(C, 1)

    in_maps = []
    for i in range(B):
        yT = np.ascontiguousarray(y[i].T)
        yh, yl = _hi_lo(yT)
        in_maps.append({
            "AT": np.asarray(AT[i], NPE4),
            "yh": yh,
            "yl": yl,
            "wvh": wvh,
            "wvl": wvl,
            "wprojT": wprojT,
            "bproj": bproj,
        })
    return in_maps
